# revision 50
# baseline (speedup 1.0000x reference)
"""Trainium2 Bass kernel for nn_LinearLayer_45243185496808.

Computes out[b,o] = sum_i tanh(x[b,i]*t) * (sum_p coef[o,i,p]) with
B=131072, I=O=128, P_NUM=16, data-parallel over batch on 8 NeuronCores.

Per-core pipeline (B_CORE=16384 rows, ~55us HW time, DMA-bound):
  - prelude: w2 = [w_T | w_T] (f32r) from a DVE pairwise-tree reduction of
    coef plus one PE transpose; 8 identity matmuls warm the PE clock (HAM).
  - x streams in as f32 but lands in SBUF as f16 (gpsimd SWDGE casts in
    flight); per 128-row slice: PE f16-transpose (1 cyc/row) -> PSUM,
    ScalarE tanh(scale*x) PSUM->SBUF as f32r, one single-pass f32r matmul
    against w2 (N=256; duplicated w reaches the full-rate regime) -> PSUM,
    VectorE strided copy -> f16 out tile, gpsimd SWDGE store (separate DGE
    ring from loads so stores never block loads).
  - output returns as f16 and is upcast to f32 on the host (exact).
Accuracy vs float64 reference: ~4.0e-4 absmax-relative (f32r matmul
~1.5e-4, f16 in/out rounding ~2^-11 each).
"""

import os
import sys
import types

import numpy as np

import concourse.bass as bass
import concourse.mybir as mybir
import concourse.tile as tile
from concourse import bacc, masks
from concourse.bass_utils import run_bass_kernel_spmd


def _ensure_ntff_hook():
    """Register the axon NTFF profile hook if the image lacks antenv.axon_hooks.

    Only needed for BASS_TRACE=1 profiling runs; harmless otherwise."""
    if "antenv.axon_hooks" in sys.modules:
        return
    try:
        from antenv.axon_hooks import get_axon_ntff_profile_hook  # noqa: F401

        return  # real module importable
    except ImportError:
        pass
    hook = None
    try:
        from trn_agent_boot.trn_boot import _ntff_profile_via_ctypes

        so_path = "/opt/axon/libaxon_pjrt.so"
        if os.path.exists(so_path):
            hook = _ntff_profile_via_ctypes(so_path)
    except Exception:
        hook = None
    mod = types.ModuleType("antenv.axon_hooks")
    mod.get_axon_ntff_profile_hook = lambda: hook
    mod.set_axon_ntff_profile_hook = lambda h: None
    sys.modules["antenv.axon_hooks"] = mod

N_CORES = 8
B_FULL = 131072
I_DIM = 128
O_DIM = 128
P_NUM = 16
P = 128                     # SBUF partitions
RPP = 16                    # max rows/partition per store piece
B_CORE = B_FULL // N_CORES  # 16384
G = 4                       # 128-row slices per PSUM-bank group

LAST_RESULT = None  # BassKernelResults of the most recent run (for test.py)


def build_bass(tanh_scale: float) -> bass.Bass:
    nc = bacc.Bacc("TRN2", target_bir_lowering=False)
    x = nc.dram_tensor("x", [B_CORE, I_DIM], mybir.dt.float32, kind="ExternalInput")
    coef = nc.dram_tensor(
        "coef", [O_DIM, I_DIM, P_NUM], mybir.dt.float32, kind="ExternalInput"
    )
    # Output leaves the device as f16 (halves store traffic; |out| << f16
    # range, adds ~2^-11 relative rounding). Host upcasts back to f32.
    out = nc.dram_tensor("out", [B_CORE, O_DIM], mybir.dt.float16, kind="ExternalOutput")

    # Chunk descriptors (row0, rows-per-partition). The small first chunk gets
    # compute started ~5us earlier; the rest stream at 2 MiB for DMA
    # efficiency. Each chunk is contiguous per partition in DRAM.
    chunk_plan = [
        (0, 4), (512, 12), (2048, 16), (4096, 32), (8192, 32), (12288, 24),
        (15360, 8),
    ]
    assert sum(r for _, r in chunk_plan) * P == B_CORE
    assert all(
        a + r * P == b for (a, r), (b, _) in zip(chunk_plan, chunk_plan[1:])
    )

    def chunk_view(t, row0, rpp):
        return t[row0 : row0 + rpp * P, :].rearrange("(p r) d -> p (r d)", p=P)

    coef_flat = coef[:, :, :].rearrange("o i p -> o (i p)")

    with tile.TileContext(nc) as tc:
        with (
            tc.tile_pool(name="consts", bufs=1) as consts,
            tc.tile_pool(name="xin", bufs=3) as xin_pool,
            tc.tile_pool(name="vals", bufs=4) as vals_pool,
            tc.tile_pool(name="outp", bufs=4) as out_pool,
            tc.tile_pool(name="pxT", bufs=4, space="PSUM") as pxT_pool,
            tc.tile_pool(name="pout", bufs=4, space="PSUM") as pout_pool,
        ):
            identity = consts.tile([P, P], mybir.dt.float32)
            masks.make_identity(nc, identity[:])
            identity_h = consts.tile([P, P], mybir.dt.float16)
            masks.make_identity(nc, identity_h[:])

            # --- prelude ---
            coef_sb = consts.tile([P, I_DIM * P_NUM], mybir.dt.float32)
            nc.sync.dma_start(out=coef_sb[:], in_=coef_flat)

            # PE warmup on the identity while the coef/x DMAs are in flight,
            # so HAM reaches K=8/8 before the real work.
            for wi in range(2):
                wm_ps = pout_pool.tile([P, G * O_DIM], mybir.dt.float32, tag="o_ps")
                for wj in range(G):
                    nc.tensor.matmul(
                        wm_ps[:, wj * P : (wj + 1) * P],
                        identity[:],
                        identity[:],
                        start=True,
                        stop=True,
                    )

            # w_oi = sum_p coef via a pairwise halving tree on DVE (4 fat
            # strided adds; p is the minor axis so pairs are adjacent).
            cur = coef_sb
            width = I_DIM * P_NUM
            while width > I_DIM:
                nxt = consts.tile(
                    [P, width // 2], mybir.dt.float32, tag=f"wred{width}"
                )
                pairs = cur[:, :width].rearrange("p (x two) -> p x two", two=2)
                nc.vector.tensor_add(nxt[:], pairs[:, :, 0], pairs[:, :, 1])
                cur = nxt
                width //= 2
            w_oi = cur
            w_ps = pout_pool.tile([P, G * O_DIM], mybir.dt.float32, tag="o_ps")
            nc.tensor.transpose(w_ps[:, :O_DIM], w_oi[:], identity[:])
            # w2 = [w_T | w_T] in float32r: duplicating w widens the moving
            # operand to N=256, where f32r matmul runs single-pass full rate
            # (fp32 runs two half-speed passes).
            w2 = consts.tile([P, 2 * O_DIM], mybir.dt.float32r)
            nc.vector.tensor_copy(w2[:, :O_DIM], w_ps[:, :O_DIM])
            nc.vector.tensor_copy(w2[:, O_DIM : 2 * O_DIM], w_ps[:, :O_DIM])

            # --- main loop ---
            # Loads on the sync HWDGE ring; stores on the gpsimd SWDGE rings
            # so they never queue behind loads. Stores go out in <=1 MiB
            # pieces as soon as their slices are done.
            for ci, (row0, rpp) in enumerate(chunk_plan):
                # gpsimd (SWDGE) DMA casts f32->f16 in flight: full-fidelity
                # 8 MiB HBM read, half-size SBUF tiles, and f16 transposes
                # run at 1 PE cycle/row instead of f32's 2. The first (tiny)
                # chunk instead takes the lower-latency HWDGE ring in f32 so
                # compute starts sooner.
                first = ci == 0
                x_dt = mybir.dt.float32 if first else mybir.dt.float16
                ident = identity if first else identity_h
                x_sb = xin_pool.tile([P, rpp * I_DIM], x_dt, tag="x_sb")
                (nc.sync if first else nc.gpsimd).dma_start(
                    out=x_sb[:], in_=chunk_view(x, row0, rpp)
                )
                out_view = chunk_view(out, row0, rpp)
                n_pieces = -(-rpp // RPP)
                piece = rpp // n_pieces
                assert piece % G == 0 and piece * n_pieces == rpp
                for pc in range(n_pieces):
                    out_sb = out_pool.tile(
                        [P, piece * O_DIM], mybir.dt.float16, tag="out_sb"
                    )
                    for g in range(piece // G):
                        xT_ps = pxT_pool.tile([P, G * P], x_dt, tag="xT_ps")
                        for j in range(G):
                            n = pc * piece + g * G + j
                            nc.tensor.transpose(
                                xT_ps[:, j * P : (j + 1) * P],
                                x_sb[:, n * I_DIM : (n + 1) * I_DIM],
                                ident[:],
                            )
                        v_T = vals_pool.tile([P, G * P], mybir.dt.float32r)
                        nc.scalar.activation(
                            v_T[:],
                            xT_ps[:],
                            mybir.ActivationFunctionType.Tanh,
                            scale=tanh_scale,
                        )
                        # Two slices per PSUM bank: each f32r matmul emits
                        # [out_j | dup_j] at N=256.
                        for half in range(G // 2):
                            o_ps = pout_pool.tile(
                                [P, 2 * 2 * O_DIM], mybir.dt.float32
                            )
                            for jj in range(2):
                                j = half * 2 + jj
                                nc.tensor.matmul(
                                    o_ps[:, jj * 2 * O_DIM : (jj + 1) * 2 * O_DIM],
                                    v_T[:, j * P : (j + 1) * P],
                                    w2[:],
                                    start=True,
                                    stop=True,
                                )
                            n0 = g * G + half * 2
                            dst = out_sb[
                                :, n0 * O_DIM : (n0 + 2) * O_DIM
                            ].rearrange("p (two o) -> p two o", two=2)
                            src = o_ps[:].rearrange(
                                "p (two o2) -> p two o2", two=2
                            )[:, :, :O_DIM]
                            nc.vector.tensor_copy(dst, src)
                    # Final store rides the HWDGE ring (lower first-byte
                    # latency; loads are finished by then).
                    last = ci == len(chunk_plan) - 1 and pc == n_pieces - 1
                    (nc.sync if last else nc.gpsimd).dma_start(
                        out=out_view[
                            :, pc * piece * O_DIM : (pc + 1) * piece * O_DIM
                        ],
                        in_=out_sb[:],
                    )
    nc.finalize()
    return nc


def kernel(x, coef, tanh_range):
    global LAST_RESULT
    x = np.ascontiguousarray(np.asarray(x, dtype=np.float32))
    coef = np.ascontiguousarray(np.asarray(coef, dtype=np.float32))
    t = float(np.asarray(tanh_range))
    assert x.shape == (B_FULL, I_DIM), x.shape
    assert coef.shape == (O_DIM, I_DIM, P_NUM), coef.shape

    nc = build_bass(t)
    in_maps = [
        {"x": np.ascontiguousarray(x[k * B_CORE : (k + 1) * B_CORE]), "coef": coef}
        for k in range(N_CORES)
    ]
    if os.environ.get("BASS_TRACE"):
        _ensure_ntff_hook()
    res = run_bass_kernel_spmd(nc, in_maps, core_ids=list(range(N_CORES)))
    LAST_RESULT = res
    return np.concatenate(
        [r["out"].astype(np.float32) for r in res.results], axis=0
    )


# revision 51
# speedup vs baseline: 1.0839x; 1.0839x over previous
"""Trainium2 Bass kernel for nn_LinearLayer_45243185496808.

Computes out[b,o] = sum_i tanh(x[b,i]*t) * (sum_p coef[o,i,p]) with
B=131072, I=O=128, P_NUM=16, data-parallel over batch on 8 NeuronCores.

Per-core pipeline (B_CORE=16384 rows, ~55us HW time, DMA-bound):
  - prelude: w2 = [w_T | w_T] (f32r) from a DVE pairwise-tree reduction of
    coef plus one PE transpose; 8 identity matmuls warm the PE clock (HAM).
  - x streams in as f32 but lands in SBUF as f16 (gpsimd SWDGE casts in
    flight); per 128-row slice: PE f16-transpose (1 cyc/row) -> PSUM,
    ScalarE tanh(scale*x) PSUM->SBUF as f32r, one single-pass f32r matmul
    against w2 (N=256; duplicated w reaches the full-rate regime) -> PSUM,
    VectorE strided copy -> f16 out tile, gpsimd SWDGE store (separate DGE
    ring from loads so stores never block loads).
  - output returns as f16 and is upcast to f32 on the host (exact).
Accuracy vs float64 reference: ~4.0e-4 absmax-relative (f32r matmul
~1.5e-4, f16 in/out rounding ~2^-11 each).
"""

import os
import sys
import types

import numpy as np

import concourse.bass as bass
import concourse.mybir as mybir
import concourse.tile as tile
from concourse import bacc, masks
from concourse.bass_utils import run_bass_kernel_spmd


def _ensure_ntff_hook():
    """Register the axon NTFF profile hook if the image lacks antenv.axon_hooks.

    Only needed for BASS_TRACE=1 profiling runs; harmless otherwise."""
    if "antenv.axon_hooks" in sys.modules:
        return
    try:
        from antenv.axon_hooks import get_axon_ntff_profile_hook  # noqa: F401

        return  # real module importable
    except ImportError:
        pass
    hook = None
    try:
        from trn_agent_boot.trn_boot import _ntff_profile_via_ctypes

        so_path = "/opt/axon/libaxon_pjrt.so"
        if os.path.exists(so_path):
            hook = _ntff_profile_via_ctypes(so_path)
    except Exception:
        hook = None
    mod = types.ModuleType("antenv.axon_hooks")
    mod.get_axon_ntff_profile_hook = lambda: hook
    mod.set_axon_ntff_profile_hook = lambda h: None
    sys.modules["antenv.axon_hooks"] = mod

N_CORES = 8
B_FULL = 131072
I_DIM = 128
O_DIM = 128
P_NUM = 16
P = 128                     # SBUF partitions
RPP = 16                    # max rows/partition per store piece
B_CORE = B_FULL // N_CORES  # 16384
G = 4                       # 128-row slices per PSUM-bank group

LAST_RESULT = None  # BassKernelResults of the most recent run (for test.py)


def build_bass(tanh_scale: float) -> bass.Bass:
    nc = bacc.Bacc("TRN2", target_bir_lowering=False)
    x = nc.dram_tensor("x", [B_CORE, I_DIM], mybir.dt.float32, kind="ExternalInput")
    coef = nc.dram_tensor(
        "coef", [O_DIM, I_DIM, P_NUM], mybir.dt.float32, kind="ExternalInput"
    )
    # Output leaves the device as f16 (halves store traffic; |out| << f16
    # range, adds ~2^-11 relative rounding). Host upcasts back to f32.
    out = nc.dram_tensor("out", [B_CORE, O_DIM], mybir.dt.float16, kind="ExternalOutput")

    # Chunk descriptors (row0, rows-per-partition). The small first chunk gets
    # compute started ~5us earlier; the rest stream at 2 MiB for DMA
    # efficiency. Each chunk is contiguous per partition in DRAM.
    chunk_plan = [
        (0, 4), (512, 12), (2048, 16), (4096, 32), (8192, 32), (12288, 24),
        (15360, 8),
    ]
    assert sum(r for _, r in chunk_plan) * P == B_CORE
    assert all(
        a + r * P == b for (a, r), (b, _) in zip(chunk_plan, chunk_plan[1:])
    )

    def chunk_view(t, row0, rpp):
        return t[row0 : row0 + rpp * P, :].rearrange("(p r) d -> p (r d)", p=P)

    coef_flat = coef[:, :, :].rearrange("o i p -> o (i p)")

    with tile.TileContext(nc) as tc:
        with (
            tc.tile_pool(name="consts", bufs=1) as consts,
            tc.tile_pool(name="xin", bufs=4) as xin_pool,
            tc.tile_pool(name="vals", bufs=4) as vals_pool,
            tc.tile_pool(name="outp", bufs=5) as out_pool,
            tc.tile_pool(name="pxT", bufs=4, space="PSUM") as pxT_pool,
            tc.tile_pool(name="pout", bufs=4, space="PSUM") as pout_pool,
        ):
            identity = consts.tile([P, P], mybir.dt.float32)
            masks.make_identity(nc, identity[:])
            identity_h = consts.tile([P, P], mybir.dt.float16)
            masks.make_identity(nc, identity_h[:])

            # --- prelude ---
            coef_sb = consts.tile([P, I_DIM * P_NUM], mybir.dt.float32)
            nc.sync.dma_start(out=coef_sb[:], in_=coef_flat)

            # PE warmup on the identity while the coef/x DMAs are in flight,
            # so HAM reaches K=8/8 before the real work.
            for wi in range(2):
                wm_ps = pout_pool.tile([P, G * O_DIM], mybir.dt.float32, tag="o_ps")
                for wj in range(G):
                    nc.tensor.matmul(
                        wm_ps[:, wj * P : (wj + 1) * P],
                        identity[:],
                        identity[:],
                        start=True,
                        stop=True,
                    )

            # w_oi = sum_p coef via a pairwise halving tree on DVE (4 fat
            # strided adds; p is the minor axis so pairs are adjacent).
            cur = coef_sb
            width = I_DIM * P_NUM
            while width > I_DIM:
                nxt = consts.tile(
                    [P, width // 2], mybir.dt.float32, tag=f"wred{width}"
                )
                pairs = cur[:, :width].rearrange("p (x two) -> p x two", two=2)
                nc.vector.tensor_add(nxt[:], pairs[:, :, 0], pairs[:, :, 1])
                cur = nxt
                width //= 2
            w_oi = cur
            w_ps = pout_pool.tile([P, G * O_DIM], mybir.dt.float32, tag="o_ps")
            nc.tensor.transpose(w_ps[:, :O_DIM], w_oi[:], identity[:])
            # w2 = [w_T | w_T] in float32r: duplicating w widens the moving
            # operand to N=256, where f32r matmul runs single-pass full rate
            # (fp32 runs two half-speed passes).
            w2 = consts.tile([P, 2 * O_DIM], mybir.dt.float32r)
            nc.vector.tensor_copy(w2[:, :O_DIM], w_ps[:, :O_DIM])
            nc.vector.tensor_copy(w2[:, O_DIM : 2 * O_DIM], w_ps[:, :O_DIM])

            # --- main loop ---
            # Loads on the sync HWDGE ring; stores on the gpsimd SWDGE rings
            # so they never queue behind loads. Stores go out in <=1 MiB
            # pieces as soon as their slices are done.
            for ci, (row0, rpp) in enumerate(chunk_plan):
                # gpsimd (SWDGE) DMA casts f32->f16 in flight: full-fidelity
                # 8 MiB HBM read, half-size SBUF tiles, and f16 transposes
                # run at 1 PE cycle/row instead of f32's 2. The first (tiny)
                # chunk instead takes the lower-latency HWDGE ring in f32 so
                # compute starts sooner.
                first = ci == 0
                x_dt = mybir.dt.float32 if first else mybir.dt.float16
                ident = identity if first else identity_h
                x_sb = xin_pool.tile([P, rpp * I_DIM], x_dt, tag="x_sb")
                (nc.sync if first else nc.gpsimd).dma_start(
                    out=x_sb[:], in_=chunk_view(x, row0, rpp)
                )
                out_view = chunk_view(out, row0, rpp)
                n_pieces = -(-rpp // RPP)
                piece = rpp // n_pieces
                assert piece % G == 0 and piece * n_pieces == rpp
                for pc in range(n_pieces):
                    out_sb = out_pool.tile(
                        [P, piece * O_DIM], mybir.dt.float16, tag="out_sb"
                    )
                    for g in range(piece // G):
                        xT_ps = pxT_pool.tile([P, G * P], x_dt, tag="xT_ps")
                        for j in range(G):
                            n = pc * piece + g * G + j
                            nc.tensor.transpose(
                                xT_ps[:, j * P : (j + 1) * P],
                                x_sb[:, n * I_DIM : (n + 1) * I_DIM],
                                ident[:],
                            )
                        v_T = vals_pool.tile([P, G * P], mybir.dt.float32r)
                        nc.scalar.activation(
                            v_T[:],
                            xT_ps[:],
                            mybir.ActivationFunctionType.Tanh,
                            scale=tanh_scale,
                        )
                        # Two slices per PSUM bank: each f32r matmul emits
                        # [out_j | dup_j] at N=256.
                        for half in range(G // 2):
                            o_ps = pout_pool.tile(
                                [P, 2 * 2 * O_DIM], mybir.dt.float32
                            )
                            for jj in range(2):
                                j = half * 2 + jj
                                nc.tensor.matmul(
                                    o_ps[:, jj * 2 * O_DIM : (jj + 1) * 2 * O_DIM],
                                    v_T[:, j * P : (j + 1) * P],
                                    w2[:],
                                    start=True,
                                    stop=True,
                                )
                            n0 = g * G + half * 2
                            dst = out_sb[
                                :, n0 * O_DIM : (n0 + 2) * O_DIM
                            ].rearrange("p (two o) -> p two o", two=2)
                            src = o_ps[:].rearrange(
                                "p (two o2) -> p two o2", two=2
                            )[:, :, :O_DIM]
                            nc.vector.tensor_copy(dst, src)
                    # Final store rides the HWDGE ring (lower first-byte
                    # latency; loads are finished by then).
                    last = ci == len(chunk_plan) - 1 and pc == n_pieces - 1
                    (nc.sync if last else nc.gpsimd).dma_start(
                        out=out_view[
                            :, pc * piece * O_DIM : (pc + 1) * piece * O_DIM
                        ],
                        in_=out_sb[:],
                    )
    nc.finalize()
    return nc


def kernel(x, coef, tanh_range):
    global LAST_RESULT
    x = np.ascontiguousarray(np.asarray(x, dtype=np.float32))
    coef = np.ascontiguousarray(np.asarray(coef, dtype=np.float32))
    t = float(np.asarray(tanh_range))
    assert x.shape == (B_FULL, I_DIM), x.shape
    assert coef.shape == (O_DIM, I_DIM, P_NUM), coef.shape

    nc = build_bass(t)
    in_maps = [
        {"x": np.ascontiguousarray(x[k * B_CORE : (k + 1) * B_CORE]), "coef": coef}
        for k in range(N_CORES)
    ]
    if os.environ.get("BASS_TRACE"):
        _ensure_ntff_hook()
    res = run_bass_kernel_spmd(nc, in_maps, core_ids=list(range(N_CORES)))
    LAST_RESULT = res
    return np.concatenate(
        [r["out"].astype(np.float32) for r in res.results], axis=0
    )
